# revision 8
# baseline (speedup 1.0000x reference)
"""Causal self-attention (B=2, T=2048, D=1024, H=16) on 8 TRN2 NeuronCores.

Sharding: 8-way tensor-parallel over heads (2 heads/core, both batches).
The head->token reshard is split into TWO AllToAlls (batch 0, then batch 1)
so the first collective and half the output projection overlap with batch-1
attention compute; slice ownership is 256-token interleaved (core r owns
tokens [256r, 256r+256) of BOTH batches).

Per-core program (SPMD, identical program, per-core data):
  core r: heads {2r, 2r+1}  -> qkv channel slice [128r : 128r+128)
          output slice      -> tokens [256r, 256r+256) of b0 and b1

x^T is pre-transposed on the host (removes all PE transposes of x), QKV
biases are folded into the PSUM evacuation (DVE tensor_scalar_add with a
per-partition bias), the causal mask is a 0/1 DVE multiply on the exp'd
probabilities (no mask matmuls), and softmax normalization uses
reciprocal_approx_fast (~5x faster than DVE reciprocal, ~18 bits).

bf16 matmul inputs, fp32 PSUM accumulation, fp32 output.

Attention is processed as (batch, 512-query-block) groups, two groups in
flight round-robin; per k-block each head's scores go to a 1-bank PSUM tile
(pss tag, 3 bufs) and one Exp per head on ScalarE. Causality is handled by
skipping above-diagonal k-blocks plus the 0/1 mask mul on the diagonal tile.
The softmax denominator comes from an appended ones-column in V';
normalization is reciprocal + GpSimd partition-broadcast.

PSUM budget: pss 3x1 bank + av (attention accumulators) 4x1 bank +
po (out-proj, dedicated tag to avoid PE-FIFO/slot-release deadlocks) 1 bank.
"""

import numpy as np
import ml_dtypes
import itertools
from contextlib import ExitStack

import concourse.bass as bass
import concourse.tile as tile
from concourse import mybir, bacc
from concourse.bass_utils import run_bass_kernel_spmd

F32 = mybir.dt.float32
BF16 = mybir.dt.bfloat16

B, T, D, H, HD = 2, 2048, 1024, 16, 64
NC = 8  # cores
TI = B * T  # token instances = 4096
SCALE = HD ** -0.5


def build_nc() -> bass.Bass:
    nc = bacc.Bacc("TRN2", target_bir_lowering=False, debug=False, num_devices=NC)

    # x^T host-packed per 512-token block: xt[g][p, 512c+t] = x[512g+t, 128c+p]
    xt = nc.dram_tensor("xt", [8, 128, TI], BF16, kind="ExternalInput").ap()
    # host-packed: wq[p, 128c+m] = Wq[128c+p, my_ch m]
    wq = nc.dram_tensor("wq", [128, D], BF16, kind="ExternalInput").ap()
    wk = nc.dram_tensor("wk", [128, D], BF16, kind="ExternalInput").ap()
    wv = nc.dram_tensor("wv", [128, D], BF16, kind="ExternalInput").ap()
    bq = nc.dram_tensor("bq", [128, 1], F32, kind="ExternalInput").ap()
    bk = nc.dram_tensor("bk", [128, 1], F32, kind="ExternalInput").ap()
    bv = nc.dram_tensor("bv", [128, 1], F32, kind="ExternalInput").ap()
    # host-packed: wo[p, 1024c+n] = Wo[128c+p, n]
    wo = nc.dram_tensor("wo", [128, 8 * D], BF16, kind="ExternalInput").ap()
    bo = nc.dram_tensor("bo", [D], BF16, kind="ExternalInput").ap()
    # 0/1 causal keep-mask (1 where k<=q), duplicated horizontally for 2 heads
    tri2 = nc.dram_tensor("tri2", [128, 256], BF16, kind="ExternalInput").ap()
    eye = nc.dram_tensor("eye", [128, 128], BF16, kind="ExternalInput").ap()
    out = nc.dram_tensor("out", [512, D], F32, kind="ExternalOutput").ap()

    with tile.TileContext(nc) as tc, ExitStack() as ctx:
        const = ctx.enter_context(tc.tile_pool(name="const", bufs=1))
        qkvp = ctx.enter_context(tc.tile_pool(name="qkvp", bufs=1))
        xtp = ctx.enter_context(tc.tile_pool(name="xtp", bufs=1))
        vtb = ctx.enter_context(tc.tile_pool(name="vtb", bufs=2))
        ptp = ctx.enter_context(tc.tile_pool(name="ptp", bufs=6))
        rp = ctx.enter_context(tc.tile_pool(name="rp", bufs=2))
        atp = ctx.enter_context(tc.tile_pool(name="atp", bufs=3))
        aoutp = ctx.enter_context(tc.tile_pool(name="aoutp", bufs=2))
        osb = ctx.enter_context(tc.tile_pool(name="osb", bufs=2))
        psS = ctx.enter_context(tc.tile_pool(name="psS", bufs=2, space="PSUM"))
        psB = ctx.enter_context(tc.tile_pool(name="psB", bufs=4, space="PSUM"))
        dram = ctx.enter_context(tc.tile_pool(name="dram", bufs=1, space="DRAM"))

        # ---- constants / weights -------------------------------------------------
        # DMA order matters: the PE's first work (batch-0 projections) needs
        # wq/wk/wv + xt blocks 0-3; everything else (wo especially, 2MB) waits.
        wq_sb = const.tile([128, D], BF16)  # col 128c+m  <- wq[128c+p, m]
        wk_sb = const.tile([128, D], BF16)
        wv_sb = const.tile([128, D], BF16)
        bq_sb = const.tile([128, 1], F32)
        bk_sb = const.tile([128, 1], F32)
        bv_sb = const.tile([128, 1], F32)
        bo_sb = const.tile([1, D], BF16)
        wo_sb = const.tile([128, 8 * D], BF16)  # col 1024c+n <- wo[128c+p, n]
        tri2_sb = const.tile([128, 256], BF16)
        eye_sb = const.tile([128, 128], BF16)
        ones_sb = const.tile([1, 512], BF16)
        xt_sb = xtp.tile([128, 8 * TI], BF16)  # col 4096g + 512c + t (g=token block)

        def load_xt_block(blk8):
            nc.sync.dma_start(
                xt_sb[:, TI * blk8 : TI * (blk8 + 1)], xt[blk8]
            )

        for w_sb, w in ((wq_sb, wq), (wk_sb, wk), (wv_sb, wv)):
            nc.sync.dma_start(w_sb[:], w[:])
        load_xt_block(0)
        nc.sync.dma_start(bq_sb[:], bq[:])
        nc.sync.dma_start(bk_sb[:], bk[:])
        nc.sync.dma_start(bv_sb[:], bv[:])
        nc.sync.dma_start(eye_sb[:], eye[:])
        load_xt_block(1)
        nc.sync.dma_start(tri2_sb[:], tri2[:])
        load_xt_block(2)
        load_xt_block(3)
        nc.vector.memset(ones_sb[:], 1.0)
        for blk8 in range(4, 8):
            load_xt_block(blk8)
        nc.sync.dma_start(bo_sb[:], bo[None, :])
        nc.sync.dma_start(wo_sb[:], wo[:])

        # Q^T | K^T packed: col t -> Q^T, col TI + t -> K^T  (channels on partitions)
        qkt_sb = qkvp.tile([128, 2 * TI], BF16)
        # V' : [kpos(128), 32 ktiles x (2 heads x 128)]; col 256*kt + 128*h + d.
        # d 0:64 are ones columns: the AV matmul then emits the softmax
        # denominator already replicated on partitions 0:64 (no partition
        # broadcast needed, keeps the Pool queue free for the collectives);
        # d 64:128 are the V values.
        vp_sb = qkvp.tile([128, 32 * 256], BF16)

        # two half-sized AllToAlls: b=0 slices, then b=1 slices.
        # rows 256*j + 128*sl + 64*h + p  (slice s=2j+sl -> dest core s)
        a2a_in = [dram.tile([1024, 256], BF16, name=f"a2a_in{b}") for b in range(2)]
        a2a_out = [dram.tile([1024, 256], BF16, name=f"a2a_out{b}") for b in range(2)]

        # ---- phase A/B: QKV projections, per 512-token block.
        # Generator of PE-sized chunks so batch 1's projection work can be
        # interleaved into batch 0's attention emission.
        def ab_block(b, blk):
            base = 2048 * b + 512 * blk
            g = 4 * b + blk

            def proj(w_sb, b_sb, dst):
                ps = psS.tile([128, 512], F32, name="ps_p", tag="pss")
                for c in range(8):
                    nc.tensor.matmul(
                        ps[:],
                        w_sb[:, 128 * c : 128 * (c + 1)],
                        xt_sb[:, 4096 * g + 512 * c : 4096 * g + 512 * (c + 1)],
                        start=(c == 0),
                        stop=(c == 7),
                    )
                # evacuate with bias folded in (per-partition scalar add)
                nc.vector.tensor_scalar_add(dst, ps[:], b_sb[:])

            proj(wq_sb, bq_sb, qkt_sb[:, base : base + 512])
            yield
            proj(wk_sb, bk_sb, qkt_sb[:, TI + base : TI + base + 512])
            yield
            vt_blk = vtb.tile([128, 512], BF16, name="vt_blk")
            proj(wv_sb, bv_sb, vt_blk[:])
            yield
            # V' tiles via PE transpose
            ps2 = psS.tile([128, 512], BF16, name="ps_vt", tag="pss")
            for i in range(4):
                nc.tensor.transpose(
                    ps2[:, 128 * i : 128 * (i + 1)],
                    vt_blk[:, 128 * i : 128 * (i + 1)],
                    eye_sb[:],
                )
            kt0 = 16 * b + 4 * blk
            blk_vp = vp_sb[:, 256 * kt0 : 256 * (kt0 + 4)].rearrange(
                "p (kt h d) -> p kt h d", kt=4, h=2, d=128
            )
            nc.vector.memset(blk_vp[:, :, :, 0:64], 1.0)
            src = ps2[:].rearrange("p (i h d) -> p i h d", i=4, h=2, d=64)
            nc.vector.tensor_copy(blk_vp[:, :, :, 64:128], src)
            yield

        # batch 0 projections emitted up front; batch 1 paced into phase C
        for blk in range(4):
            for _ in ab_block(0, blk):
                pass
        ab1 = itertools.chain.from_iterable(ab_block(1, blk) for blk in range(4))

        # ---- out-projection pass for one 256-token half (after a2a b) ------------
        def outproj_pass(b):
            attn2 = aoutp.tile([128, 8 * 256], BF16, name="attn2")  # col 256c+t
            for c in range(8):
                nc.sync.dma_start(
                    attn2[:, 256 * c : 256 * (c + 1)],
                    a2a_out[b][128 * c : 128 * (c + 1), :],
                )
            for mt in range(2):
                for nh in range(2):
                    sl = slice(512 * nh, 512 * (nh + 1))
                    po = psB.tile([128, 512], F32, name="ps_o", tag="av")
                    for c in range(8):
                        nc.tensor.matmul(
                            po[:],
                            attn2[:, 256 * c + 128 * mt : 256 * c + 128 * (mt + 1)],
                            wo_sb[:, 1024 * c + 512 * nh : 1024 * c + 512 * (nh + 1)],
                            start=(c == 0),
                            stop=False,
                        )
                    nc.tensor.matmul(
                        po[:], ones_sb[:, 0:128], bo_sb[:, sl], start=False, stop=True
                    )
                    o_t = osb.tile([128, 512], F32, name="o_t")
                    nc.vector.tensor_copy(o_t[:], po[:])
                    nc.sync.dma_start(
                        out[256 * b + 128 * mt : 256 * b + 128 * (mt + 1), sl], o_t[:]
                    )
                    yield

        # ---- phase C: attention; two (batch, q-block) groups in flight -----------
        class Group:
            def __init__(self, b, j):
                self.b, self.j = b, j
                self.nkb = 4 * j + 4
                self.kb_s = 0  # next k-block to score
                self.kb_a = 0  # next k-block to accumulate into AV
                self.qbase = 2048 * b + 512 * j
                self.avs = [
                    psB.tile([128, 512], F32, name=f"av{h}", tag="av")
                    for h in range(2)
                ]
                self.pts = {}

            def emit_scores(self):
                kb = self.kb_s
                self.kb_s += 1
                m = kb - 4 * self.j
                off = 128 * m if m >= 0 else 0
                kbase = TI + 2048 * self.b + 128 * kb
                pt = ptp.tile([128, 1024], BF16, name="pt")
                ps_s = psS.tile([128, 1024], F32, name="ps_s", tag="pss")
                for h in range(2):
                    hr = 64 * h
                    nc.tensor.matmul(
                        ps_s[:, 512 * h + off : 512 * (h + 1)],
                        qkt_sb[hr : hr + 64, kbase : kbase + 128],
                        qkt_sb[hr : hr + 64, self.qbase + off : self.qbase + 512],
                        start=True,
                        stop=True,
                    )
                # one Exp covers both heads (strided view over the 2 banks)
                pt_v = pt.rearrange("p (s t) -> p s t", s=2)[:, :, off:512]
                ps_v = ps_s.rearrange("p (s t) -> p s t", s=2)[:, :, off:512]
                nc.scalar.activation(
                    pt_v, ps_v, mybir.ActivationFunctionType.Exp, scale=SCALE
                )
                if m >= 0:
                    # zero the strictly-upper triangle of the diagonal
                    # 128x128 tile (both heads in one DVE op)
                    ptd = pt.rearrange("p (s t) -> p s t", s=2)[:, :, off : off + 128]
                    nc.vector.tensor_mul(
                        ptd, ptd, tri2_sb[:].rearrange("p (s t) -> p s t", s=2)
                    )
                self.pts[kb] = (pt, off)

            def emit_av(self):
                kb = self.kb_a
                self.kb_a += 1
                pt, off = self.pts.pop(kb)
                vb = 256 * (16 * self.b + kb)
                for h in range(2):
                    nc.tensor.matmul(
                        self.avs[h][:, off:],
                        vp_sb[:, vb + 128 * h : vb + 128 * h + 128],
                        pt[:, 512 * h + off : 512 * (h + 1)],
                        start=(kb == 0),
                        stop=(kb == self.nkb - 1),
                    )

            def finalize(self):
                for h in range(2):
                    # avs rows 0:64 hold the denominator replicated (ones
                    # columns of V'); base partition 0 so the custom DVE
                    # reciprocal reads the right partitions
                    rec = rp.tile([64, 512], F32, name="rec")
                    nc.vector.reciprocal_approx_fast(rec[:], self.avs[h][0:64, :])
                    at = atp.tile([64, 512], BF16, name="at")
                    nc.vector.tensor_mul(at[:], self.avs[h][64:128, :], rec[:])
                    dst = a2a_in[self.b].rearrange(
                        "(j sl h p) q -> j h p sl q", j=4, sl=2, h=2, p=64
                    )[self.j, h]
                    nc.sync.dma_start(
                        dst, at[:].rearrange("p (sl q) -> p sl q", sl=2)
                    )

        def emit_a2a(b):
            nc.gpsimd.collective_compute(
                "AllToAll",
                mybir.AluOpType.bypass,
                replica_groups=[list(range(NC))],
                ins=[a2a_in[b].opt()],
                outs=[a2a_out[b].opt()],
            )

        # long groups first so two groups stay in flight most of the time
        queue = [(0, 3), (0, 2), (0, 1), (0, 0), (1, 3), (1, 2), (1, 1), (1, 0)]
        active = []
        state = {"ab1_done": False, "b0_left": 4}

        def pace_ab1(n=1):
            if state["ab1_done"]:
                return
            for _ in range(n):
                if next(ab1, "end") == "end":
                    state["ab1_done"] = True
                    return

        while queue or active:
            while len(active) < 2 and queue:
                if queue[0][0] == 1:
                    pace_ab1(1000)  # batch-1 group: its inputs must be emitted
                g = Group(*queue.pop(0))
                g.emit_scores()
                active.append(g)
            for g in list(active):
                if g.kb_s < g.nkb:
                    g.emit_scores()
                pace_ab1(1)
                # AV trails scores by 2 k-blocks so the exp always has slack
                # before the PE consumes it (drain once scores are exhausted)
                if g.kb_a < g.kb_s - 1 or (g.kb_s == g.nkb and g.kb_a < g.nkb):
                    g.emit_av()
                if g.kb_a == g.nkb:
                    g.finalize()
                    active.remove(g)
                    if g.b == 0:
                        state["b0_left"] -= 1
                        if state["b0_left"] == 0:
                            emit_a2a(0)  # overlaps batch-1 attention
        pace_ab1(1000)

        # ---- tail: emit the second reshard FIRST (its queue-counter waits must
        # not include pass-0's DMAs), then pass-0 compute overlaps its transfer
        emit_a2a(1)
        for _ in outproj_pass(0):
            pass
        for _ in outproj_pass(1):
            pass

    nc.compile()
    return nc


_NC_CACHE = None


def _get_nc():
    global _NC_CACHE
    if _NC_CACHE is None:
        _NC_CACHE = build_nc()
    return _NC_CACHE


def _b16(a):
    return np.ascontiguousarray(np.asarray(a, np.float32).astype(ml_dtypes.bfloat16))


def make_in_maps(x, Wq, bq, Wk, bk, Wv, bv, Wo, bo):
    xf = np.asarray(x, np.float32).reshape(TI, D)
    # [g, p, c, t]: xt[g][p, 512c+t] = x[512g+t, 128c+p]
    xt = _b16(
        xf.reshape(8, 512, 8, 128).transpose(0, 3, 2, 1).reshape(8, 128, TI)
    )
    Wq, Wk, Wv, Wo = _b16(Wq), _b16(Wk), _b16(Wv), _b16(Wo)
    bo16 = _b16(bo)
    bqf = np.asarray(bq, np.float32).reshape(D, 1)
    bkf = np.asarray(bk, np.float32).reshape(D, 1)
    bvf = np.asarray(bv, np.float32).reshape(D, 1)
    # 0/1 keep mask (1 where k<=q), duplicated for both heads
    tri01 = np.where(np.arange(128)[:, None] <= np.arange(128)[None, :], 1.0, 0.0)
    tri2 = np.ascontiguousarray(
        np.concatenate([tri01, tri01], axis=1).astype(ml_dtypes.bfloat16)
    )
    eye = np.eye(128, dtype=ml_dtypes.bfloat16)
    def pack_w(W):  # [1024, 128] -> [128, 1024]: out[p, 128c+m] = W[128c+p, m]
        return np.ascontiguousarray(
            W.reshape(8, 128, 128).transpose(1, 0, 2).reshape(128, 1024)
        )

    wo_p = np.ascontiguousarray(  # [128, 8192]: out[p, 1024c+n] = Wo[128c+p, n]
        Wo.reshape(8, 128, 1024).transpose(1, 0, 2).reshape(128, 8192)
    )
    in_maps = []
    for r in range(NC):
        ch = slice(128 * r, 128 * (r + 1))
        in_maps.append(
            {
                "xt": xt,
                "wq": pack_w(Wq[:, ch]),
                "wk": pack_w(Wk[:, ch]),
                "wv": pack_w(Wv[:, ch]),
                "bq": np.ascontiguousarray(bqf[ch]),
                "bk": np.ascontiguousarray(bkf[ch]),
                "bv": np.ascontiguousarray(bvf[ch]),
                "wo": wo_p,
                "bo": bo16,
                "tri2": tri2,
                "eye": eye,
            }
        )
    return in_maps


def assemble(results):
    out = np.empty((B, T, D), np.float32)
    for r in range(NC):
        res = results[r]["out"]
        out[0, 256 * r : 256 * (r + 1), :] = res[0:256]
        out[1, 256 * r : 256 * (r + 1), :] = res[256:512]
    return out


def run(inputs, trace=False, **kw):
    nc = _get_nc()
    in_maps = make_in_maps(**inputs)
    res = run_bass_kernel_spmd(nc, in_maps, core_ids=list(range(NC)), trace=trace, **kw)
    return assemble(res.results), res


def kernel(**inputs) -> np.ndarray:
    out, _ = run(inputs)
    return out


# revision 9
# speedup vs baseline: 1.0074x; 1.0074x over previous
"""Causal self-attention (B=2, T=2048, D=1024, H=16) on 8 TRN2 NeuronCores.

Sharding: 8-way tensor-parallel over heads (2 heads/core, both batches).
The head->token reshard is split into TWO AllToAlls (batch 0, then batch 1)
so the first collective and half the output projection overlap with batch-1
attention compute; slice ownership is 256-token interleaved (core r owns
tokens [256r, 256r+256) of BOTH batches).

Per-core program (SPMD, identical program, per-core data):
  core r: heads {2r, 2r+1}  -> qkv channel slice [128r : 128r+128)
          output slice      -> tokens [256r, 256r+256) of b0 and b1

x^T is pre-transposed on the host (removes all PE transposes of x), QKV
biases are folded into the PSUM evacuation (DVE tensor_scalar_add with a
per-partition bias), the causal mask is a 0/1 DVE multiply on the exp'd
probabilities (no mask matmuls), and softmax normalization uses
reciprocal_approx_fast (~5x faster than DVE reciprocal, ~18 bits).

bf16 matmul inputs, fp32 PSUM accumulation, fp32 output.

Attention is processed as (batch, 512-query-block) groups, two groups in
flight round-robin; per k-block each head's scores go to a 1-bank PSUM tile
(pss tag, 3 bufs) and one Exp per head on ScalarE. Causality is handled by
skipping above-diagonal k-blocks plus the 0/1 mask mul on the diagonal tile.
The softmax denominator comes from an appended ones-column in V';
normalization is reciprocal + GpSimd partition-broadcast.

PSUM budget: pss 3x1 bank + av (attention accumulators) 4x1 bank +
po (out-proj, dedicated tag to avoid PE-FIFO/slot-release deadlocks) 1 bank.
"""

import numpy as np
import ml_dtypes
import itertools
from contextlib import ExitStack

import concourse.bass as bass
import concourse.tile as tile
from concourse import mybir, bacc
from concourse.bass_utils import run_bass_kernel_spmd

F32 = mybir.dt.float32
BF16 = mybir.dt.bfloat16

B, T, D, H, HD = 2, 2048, 1024, 16, 64
NC = 8  # cores
TI = B * T  # token instances = 4096
SCALE = HD ** -0.5


def build_nc() -> bass.Bass:
    nc = bacc.Bacc("TRN2", target_bir_lowering=False, debug=False, num_devices=NC)

    # x^T host-packed per 512-token block: xt[g][p, 512c+t] = x[512g+t, 128c+p]
    xt = nc.dram_tensor("xt", [8, 128, TI], BF16, kind="ExternalInput").ap()
    # host-packed: wq[p, 128c+m] = Wq[128c+p, my_ch m]
    wq = nc.dram_tensor("wq", [128, D], BF16, kind="ExternalInput").ap()
    wk = nc.dram_tensor("wk", [128, D], BF16, kind="ExternalInput").ap()
    wv = nc.dram_tensor("wv", [128, D], BF16, kind="ExternalInput").ap()
    bq = nc.dram_tensor("bq", [128, 1], F32, kind="ExternalInput").ap()
    bk = nc.dram_tensor("bk", [128, 1], F32, kind="ExternalInput").ap()
    bv = nc.dram_tensor("bv", [128, 1], F32, kind="ExternalInput").ap()
    # host-packed: wo[p, 1024c+n] = Wo[128c+p, n]
    wo = nc.dram_tensor("wo", [128, 8 * D], BF16, kind="ExternalInput").ap()
    bo = nc.dram_tensor("bo", [1, D], F32, kind="ExternalInput").ap()
    # 0/1 causal keep-mask (1 where k<=q), duplicated horizontally for 2 heads
    tri2 = nc.dram_tensor("tri2", [128, 256], BF16, kind="ExternalInput").ap()
    eye = nc.dram_tensor("eye", [128, 128], BF16, kind="ExternalInput").ap()
    out = nc.dram_tensor("out", [512, D], F32, kind="ExternalOutput").ap()

    with tile.TileContext(nc) as tc, ExitStack() as ctx:
        const = ctx.enter_context(tc.tile_pool(name="const", bufs=1))
        qkvp = ctx.enter_context(tc.tile_pool(name="qkvp", bufs=1))
        xtp = ctx.enter_context(tc.tile_pool(name="xtp", bufs=1))
        vtb = ctx.enter_context(tc.tile_pool(name="vtb", bufs=2))
        ptp = ctx.enter_context(tc.tile_pool(name="ptp", bufs=6))
        rp = ctx.enter_context(tc.tile_pool(name="rp", bufs=2))
        atp = ctx.enter_context(tc.tile_pool(name="atp", bufs=3))
        aoutp = ctx.enter_context(tc.tile_pool(name="aoutp", bufs=2))
        osb = ctx.enter_context(tc.tile_pool(name="osb", bufs=2))
        psS = ctx.enter_context(tc.tile_pool(name="psS", bufs=2, space="PSUM"))
        psB = ctx.enter_context(tc.tile_pool(name="psB", bufs=4, space="PSUM"))
        dram = ctx.enter_context(tc.tile_pool(name="dram", bufs=1, space="DRAM"))

        # ---- constants / weights -------------------------------------------------
        # DMA order matters: the PE's first work (batch-0 projections) needs
        # wq/wk/wv + xt blocks 0-3; everything else (wo especially, 2MB) waits.
        wq_sb = const.tile([128, D], BF16)  # col 128c+m  <- wq[128c+p, m]
        wk_sb = const.tile([128, D], BF16)
        wv_sb = const.tile([128, D], BF16)
        bq_sb = const.tile([128, 1], F32)
        bk_sb = const.tile([128, 1], F32)
        bv_sb = const.tile([128, 1], F32)
        bo_sb = const.tile([1, D], F32)
        bo_bc = const.tile([128, D], F32)  # bo broadcast to all partitions
        wo_sb = const.tile([128, 8 * D], BF16)  # col 1024c+n <- wo[128c+p, n]
        tri2_sb = const.tile([128, 256], BF16)
        eye_sb = const.tile([128, 128], BF16)
        xt_sb = xtp.tile([128, 8 * TI], BF16)  # col 4096g + 512c + t (g=token block)

        def load_xt_block(blk8):
            nc.sync.dma_start(
                xt_sb[:, TI * blk8 : TI * (blk8 + 1)], xt[blk8]
            )

        for w_sb, w in ((wq_sb, wq), (wk_sb, wk), (wv_sb, wv)):
            nc.sync.dma_start(w_sb[:], w[:])
        load_xt_block(0)
        nc.sync.dma_start(bq_sb[:], bq[:])
        nc.sync.dma_start(bk_sb[:], bk[:])
        nc.sync.dma_start(bv_sb[:], bv[:])
        nc.sync.dma_start(eye_sb[:], eye[:])
        load_xt_block(1)
        nc.sync.dma_start(tri2_sb[:], tri2[:])
        load_xt_block(2)
        load_xt_block(3)
        for blk8 in range(4, 8):
            load_xt_block(blk8)
        nc.sync.dma_start(bo_sb[:], bo[:])
        nc.gpsimd.partition_broadcast(bo_bc[:], bo_sb[:])
        nc.sync.dma_start(wo_sb[:], wo[:])

        # Q^T | K^T packed: col t -> Q^T, col TI + t -> K^T  (channels on partitions)
        qkt_sb = qkvp.tile([128, 2 * TI], BF16)
        # V' : [kpos(128), 32 ktiles x (2 heads x 128)]; col 256*kt + 128*h + d.
        # d 0:64 are ones columns: the AV matmul then emits the softmax
        # denominator already replicated on partitions 0:64 (no partition
        # broadcast needed, keeps the Pool queue free for the collectives);
        # d 64:128 are the V values.
        vp_sb = qkvp.tile([128, 32 * 256], BF16)

        # two half-sized AllToAlls: b=0 slices, then b=1 slices.
        # rows 256*j + 128*sl + 64*h + p  (slice s=2j+sl -> dest core s)
        a2a_in = [
            dram.tile([1024, 256], BF16, name=f"a2a_in{b}", tag=f"a2a_in{b}")
            for b in range(2)
        ]
        a2a_out = [
            dram.tile([1024, 256], BF16, name=f"a2a_out{b}", tag=f"a2a_out{b}")
            for b in range(2)
        ]

        # ---- phase A/B: QKV projections, per 512-token block.
        # Generator of PE-sized chunks so batch 1's projection work can be
        # interleaved into batch 0's attention emission.
        def ab_block(b, blk):
            base = 2048 * b + 512 * blk
            g = 4 * b + blk

            def proj(w_sb, b_sb, dst):
                ps = psS.tile([128, 512], F32, name="ps_p", tag="pss")
                for c in range(8):
                    nc.tensor.matmul(
                        ps[:],
                        w_sb[:, 128 * c : 128 * (c + 1)],
                        xt_sb[:, 4096 * g + 512 * c : 4096 * g + 512 * (c + 1)],
                        start=(c == 0),
                        stop=(c == 7),
                    )
                # evacuate with bias folded in (per-partition scalar add)
                nc.vector.tensor_scalar_add(dst, ps[:], b_sb[:])

            proj(wq_sb, bq_sb, qkt_sb[:, base : base + 512])
            yield
            proj(wk_sb, bk_sb, qkt_sb[:, TI + base : TI + base + 512])
            yield
            vt_blk = vtb.tile([128, 512], BF16, name="vt_blk")
            proj(wv_sb, bv_sb, vt_blk[:])
            yield
            # V' tiles via PE transpose
            ps2 = psS.tile([128, 512], BF16, name="ps_vt", tag="pss")
            for i in range(4):
                nc.tensor.transpose(
                    ps2[:, 128 * i : 128 * (i + 1)],
                    vt_blk[:, 128 * i : 128 * (i + 1)],
                    eye_sb[:],
                )
            kt0 = 16 * b + 4 * blk
            blk_vp = vp_sb[:, 256 * kt0 : 256 * (kt0 + 4)].rearrange(
                "p (kt h d) -> p kt h d", kt=4, h=2, d=128
            )
            nc.vector.memset(blk_vp[:, :, :, 0:64], 1.0)
            src = ps2[:].rearrange("p (i h d) -> p i h d", i=4, h=2, d=64)
            nc.vector.tensor_copy(blk_vp[:, :, :, 64:128], src)
            yield

        # batch 0 projections emitted up front; batch 1 paced into phase C
        for blk in range(4):
            for _ in ab_block(0, blk):
                pass
        ab1 = itertools.chain.from_iterable(ab_block(1, blk) for blk in range(4))

        # ---- out-projection pass for one 256-token half (after a2a b) ------------
        def outproj_pass(b):
            attn2 = aoutp.tile([128, 8 * 256], BF16, name="attn2")  # col 256c+t
            for c in range(8):
                nc.sync.dma_start(
                    attn2[:, 256 * c : 256 * (c + 1)],
                    a2a_out[b][128 * c : 128 * (c + 1), :],
                )
            for mt in range(2):
                for nh in range(2):
                    sl = slice(512 * nh, 512 * (nh + 1))
                    po = psB.tile([128, 512], F32, name="ps_o", tag="av")
                    for c in range(8):
                        nc.tensor.matmul(
                            po[:],
                            attn2[:, 256 * c + 128 * mt : 256 * c + 128 * (mt + 1)],
                            wo_sb[:, 1024 * c + 512 * nh : 1024 * c + 512 * (nh + 1)],
                            start=(c == 0),
                            stop=(c == 7),
                        )
                    o_t = osb.tile([128, 512], F32, name="o_t")
                    nc.vector.tensor_add(o_t[:], po[:], bo_bc[:, sl])
                    nc.sync.dma_start(
                        out[256 * b + 128 * mt : 256 * b + 128 * (mt + 1), sl], o_t[:]
                    )
                    yield

        # ---- phase C: attention; two (batch, q-block) groups in flight -----------
        class Group:
            def __init__(self, b, j):
                self.b, self.j = b, j
                self.nkb = 4 * j + 4
                self.kb_s = 0  # next k-block to score
                self.kb_a = 0  # next k-block to accumulate into AV
                self.qbase = 2048 * b + 512 * j
                self.avs = [
                    psB.tile([128, 512], F32, name=f"av{h}", tag="av")
                    for h in range(2)
                ]
                self.pts = {}

            def emit_scores(self):
                kb = self.kb_s
                self.kb_s += 1
                m = kb - 4 * self.j
                off = 128 * m if m >= 0 else 0
                kbase = TI + 2048 * self.b + 128 * kb
                pt = ptp.tile([128, 1024], BF16, name="pt")
                ps_s = psS.tile([128, 1024], F32, name="ps_s", tag="pss")
                for h in range(2):
                    hr = 64 * h
                    nc.tensor.matmul(
                        ps_s[:, 512 * h + off : 512 * (h + 1)],
                        qkt_sb[hr : hr + 64, kbase : kbase + 128],
                        qkt_sb[hr : hr + 64, self.qbase + off : self.qbase + 512],
                        start=True,
                        stop=True,
                    )
                # one Exp covers both heads (strided view over the 2 banks)
                pt_v = pt.rearrange("p (s t) -> p s t", s=2)[:, :, off:512]
                ps_v = ps_s.rearrange("p (s t) -> p s t", s=2)[:, :, off:512]
                nc.scalar.activation(
                    pt_v, ps_v, mybir.ActivationFunctionType.Exp, scale=SCALE
                )
                if m >= 0:
                    # zero the strictly-upper triangle of the diagonal
                    # 128x128 tile (both heads in one DVE op)
                    ptd = pt.rearrange("p (s t) -> p s t", s=2)[:, :, off : off + 128]
                    nc.vector.tensor_mul(
                        ptd, ptd, tri2_sb[:].rearrange("p (s t) -> p s t", s=2)
                    )
                self.pts[kb] = (pt, off)

            def emit_av(self):
                kb = self.kb_a
                self.kb_a += 1
                pt, off = self.pts.pop(kb)
                vb = 256 * (16 * self.b + kb)
                for h in range(2):
                    nc.tensor.matmul(
                        self.avs[h][:, off:],
                        vp_sb[:, vb + 128 * h : vb + 128 * h + 128],
                        pt[:, 512 * h + off : 512 * (h + 1)],
                        start=(kb == 0),
                        stop=(kb == self.nkb - 1),
                    )

            def finalize(self):
                for h in range(2):
                    # avs rows 0:64 hold the denominator replicated (ones
                    # columns of V'); base partition 0 so the custom DVE
                    # reciprocal reads the right partitions
                    rec = rp.tile([64, 512], F32, name="rec")
                    nc.vector.reciprocal_approx_fast(rec[:], self.avs[h][0:64, :])
                    at = atp.tile([64, 512], BF16, name="at")
                    nc.vector.tensor_mul(at[:], self.avs[h][64:128, :], rec[:])
                    dst = a2a_in[self.b].rearrange(
                        "(j sl h p) q -> j h p sl q", j=4, sl=2, h=2, p=64
                    )[self.j, h]
                    nc.sync.dma_start(
                        dst, at[:].rearrange("p (sl q) -> p sl q", sl=2)
                    )

        def emit_a2a(b):
            nc.gpsimd.collective_compute(
                "AllToAll",
                mybir.AluOpType.bypass,
                replica_groups=[list(range(NC))],
                ins=[a2a_in[b].opt()],
                outs=[a2a_out[b].opt()],
            )

        # long groups first so two groups stay in flight most of the time
        queue = [(0, 3), (0, 2), (0, 1), (0, 0), (1, 3), (1, 2), (1, 1), (1, 0)]
        active = []
        state = {"ab1_done": False, "b0_left": 4}

        def pace_ab1(n=1):
            if state["ab1_done"]:
                return
            for _ in range(n):
                if next(ab1, "end") == "end":
                    state["ab1_done"] = True
                    return

        while queue or active:
            while len(active) < 2 and queue:
                if queue[0][0] == 1:
                    pace_ab1(1000)  # batch-1 group: its inputs must be emitted
                g = Group(*queue.pop(0))
                g.emit_scores()
                active.append(g)
            for g in list(active):
                if g.kb_s < g.nkb:
                    g.emit_scores()
                pace_ab1(1)
                # AV trails scores by 2 k-blocks so the exp always has slack
                # before the PE consumes it (drain once scores are exhausted)
                if g.kb_a < g.kb_s - 1 or (g.kb_s == g.nkb and g.kb_a < g.nkb):
                    g.emit_av()
                if g.kb_a == g.nkb:
                    g.finalize()
                    active.remove(g)
                    if g.b == 0:
                        state["b0_left"] -= 1
                        if state["b0_left"] == 0:
                            emit_a2a(0)  # overlaps batch-1 attention
        pace_ab1(1000)

        # ---- tail: emit the second reshard FIRST (its queue-counter waits must
        # not include pass-0's DMAs), then pass-0 compute overlaps its transfer
        emit_a2a(1)
        for _ in outproj_pass(0):
            pass
        for _ in outproj_pass(1):
            pass

    nc.compile()
    return nc


_NC_CACHE = None


def _get_nc():
    global _NC_CACHE
    if _NC_CACHE is None:
        _NC_CACHE = build_nc()
    return _NC_CACHE


def _b16(a):
    return np.ascontiguousarray(np.asarray(a, np.float32).astype(ml_dtypes.bfloat16))


def make_in_maps(x, Wq, bq, Wk, bk, Wv, bv, Wo, bo):
    xf = np.asarray(x, np.float32).reshape(TI, D)
    # [g, p, c, t]: xt[g][p, 512c+t] = x[512g+t, 128c+p]
    xt = _b16(
        xf.reshape(8, 512, 8, 128).transpose(0, 3, 2, 1).reshape(8, 128, TI)
    )
    Wq, Wk, Wv, Wo = _b16(Wq), _b16(Wk), _b16(Wv), _b16(Wo)
    bof = np.ascontiguousarray(np.asarray(bo, np.float32).reshape(1, D))
    bqf = np.asarray(bq, np.float32).reshape(D, 1)
    bkf = np.asarray(bk, np.float32).reshape(D, 1)
    bvf = np.asarray(bv, np.float32).reshape(D, 1)
    # 0/1 keep mask (1 where k<=q), duplicated for both heads
    tri01 = np.where(np.arange(128)[:, None] <= np.arange(128)[None, :], 1.0, 0.0)
    tri2 = np.ascontiguousarray(
        np.concatenate([tri01, tri01], axis=1).astype(ml_dtypes.bfloat16)
    )
    eye = np.eye(128, dtype=ml_dtypes.bfloat16)
    def pack_w(W):  # [1024, 128] -> [128, 1024]: out[p, 128c+m] = W[128c+p, m]
        return np.ascontiguousarray(
            W.reshape(8, 128, 128).transpose(1, 0, 2).reshape(128, 1024)
        )

    wo_p = np.ascontiguousarray(  # [128, 8192]: out[p, 1024c+n] = Wo[128c+p, n]
        Wo.reshape(8, 128, 1024).transpose(1, 0, 2).reshape(128, 8192)
    )
    in_maps = []
    for r in range(NC):
        ch = slice(128 * r, 128 * (r + 1))
        in_maps.append(
            {
                "xt": xt,
                "wq": pack_w(Wq[:, ch]),
                "wk": pack_w(Wk[:, ch]),
                "wv": pack_w(Wv[:, ch]),
                "bq": np.ascontiguousarray(bqf[ch]),
                "bk": np.ascontiguousarray(bkf[ch]),
                "bv": np.ascontiguousarray(bvf[ch]),
                "wo": wo_p,
                "bo": bof,
                "tri2": tri2,
                "eye": eye,
            }
        )
    return in_maps


def assemble(results):
    out = np.empty((B, T, D), np.float32)
    for r in range(NC):
        res = results[r]["out"]
        out[0, 256 * r : 256 * (r + 1), :] = res[0:256]
        out[1, 256 * r : 256 * (r + 1), :] = res[256:512]
    return out


def run(inputs, trace=False, **kw):
    nc = _get_nc()
    in_maps = make_in_maps(**inputs)
    res = run_bass_kernel_spmd(nc, in_maps, core_ids=list(range(NC)), trace=trace, **kw)
    return assemble(res.results), res


def kernel(**inputs) -> np.ndarray:
    out, _ = run(inputs)
    return out


# revision 10
# speedup vs baseline: 1.0101x; 1.0027x over previous
"""Causal self-attention (B=2, T=2048, D=1024, H=16) on 8 TRN2 NeuronCores.

Sharding: 8-way tensor-parallel over heads (2 heads/core, both batches).
The head->token reshard is split into TWO AllToAlls (batch 0, then batch 1)
so the first collective and half the output projection overlap with batch-1
attention compute; slice ownership is 256-token interleaved (core r owns
tokens [256r, 256r+256) of BOTH batches).

Per-core program (SPMD, identical program, per-core data):
  core r: heads {2r, 2r+1}  -> qkv channel slice [128r : 128r+128)
          output slice      -> tokens [256r, 256r+256) of b0 and b1

x^T is pre-transposed on the host (removes all PE transposes of x), QKV
biases are folded into the PSUM evacuation (DVE tensor_scalar_add with a
per-partition bias), the causal mask is a 0/1 DVE multiply on the exp'd
probabilities (no mask matmuls), and softmax normalization uses
reciprocal_approx_fast (~5x faster than DVE reciprocal, ~18 bits).

bf16 matmul inputs, fp32 PSUM accumulation, fp32 output.

Attention is processed as (batch, 512-query-block) groups, two groups in
flight round-robin; per k-block each head's scores go to a 1-bank PSUM tile
(pss tag, 3 bufs) and one Exp per head on ScalarE. Causality is handled by
skipping above-diagonal k-blocks plus the 0/1 mask mul on the diagonal tile.
The softmax denominator comes from an appended ones-column in V';
normalization is reciprocal + GpSimd partition-broadcast.

PSUM budget: pss 3x1 bank + av (attention accumulators) 4x1 bank +
po (out-proj, dedicated tag to avoid PE-FIFO/slot-release deadlocks) 1 bank.
"""

import numpy as np
import ml_dtypes
import itertools
from contextlib import ExitStack

import concourse.bass as bass
import concourse.tile as tile
from concourse import mybir, bacc
from concourse.bass_utils import run_bass_kernel_spmd

F32 = mybir.dt.float32
BF16 = mybir.dt.bfloat16

B, T, D, H, HD = 2, 2048, 1024, 16, 64
NC = 8  # cores
TI = B * T  # token instances = 4096
SCALE = HD ** -0.5


def build_nc() -> bass.Bass:
    nc = bacc.Bacc("TRN2", target_bir_lowering=False, debug=False, num_devices=NC)

    # x^T host-packed per 512-token block: xt[g][p, 512c+t] = x[512g+t, 128c+p]
    xt = nc.dram_tensor("xt", [8, 128, TI], BF16, kind="ExternalInput").ap()
    # host-packed: wq[p, 128c+m] = Wq[128c+p, my_ch m]
    wq = nc.dram_tensor("wq", [128, D], BF16, kind="ExternalInput").ap()
    wk = nc.dram_tensor("wk", [128, D], BF16, kind="ExternalInput").ap()
    wv = nc.dram_tensor("wv", [128, D], BF16, kind="ExternalInput").ap()
    bq = nc.dram_tensor("bq", [128, 1], F32, kind="ExternalInput").ap()
    bk = nc.dram_tensor("bk", [128, 1], F32, kind="ExternalInput").ap()
    bv = nc.dram_tensor("bv", [128, 1], F32, kind="ExternalInput").ap()
    # host-packed: wo[p, 1024c+n] = Wo[128c+p, n]
    wo = nc.dram_tensor("wo", [128, 8 * D], BF16, kind="ExternalInput").ap()
    bo = nc.dram_tensor("bo", [1, D], F32, kind="ExternalInput").ap()
    # 0/1 causal keep-mask (1 where k<=q), duplicated horizontally for 2 heads
    tri2 = nc.dram_tensor("tri2", [128, 256], BF16, kind="ExternalInput").ap()
    eye = nc.dram_tensor("eye", [128, 128], BF16, kind="ExternalInput").ap()
    out = nc.dram_tensor("out", [512, D], F32, kind="ExternalOutput").ap()

    with tile.TileContext(nc) as tc, ExitStack() as ctx:
        const = ctx.enter_context(tc.tile_pool(name="const", bufs=1))
        qkvp = ctx.enter_context(tc.tile_pool(name="qkvp", bufs=1))
        xtp = ctx.enter_context(tc.tile_pool(name="xtp", bufs=1))
        vtb = ctx.enter_context(tc.tile_pool(name="vtb", bufs=2))
        ptp = ctx.enter_context(tc.tile_pool(name="ptp", bufs=6))
        rp = ctx.enter_context(tc.tile_pool(name="rp", bufs=2))
        atp = ctx.enter_context(tc.tile_pool(name="atp", bufs=3))
        aoutp = ctx.enter_context(tc.tile_pool(name="aoutp", bufs=2))
        osb = ctx.enter_context(tc.tile_pool(name="osb", bufs=2))
        psS = ctx.enter_context(tc.tile_pool(name="psS", bufs=2, space="PSUM"))
        psB = ctx.enter_context(tc.tile_pool(name="psB", bufs=4, space="PSUM"))
        dram = ctx.enter_context(tc.tile_pool(name="dram", bufs=1, space="DRAM"))

        # ---- constants / weights -------------------------------------------------
        # DMA order matters: the PE's first work (batch-0 projections) needs
        # wq/wk/wv + xt blocks 0-3; everything else (wo especially, 2MB) waits.
        wq_sb = const.tile([128, D], BF16)  # col 128c+m  <- wq[128c+p, m]
        wk_sb = const.tile([128, D], BF16)
        wv_sb = const.tile([128, D], BF16)
        bq_sb = const.tile([128, 1], F32)
        bk_sb = const.tile([128, 1], F32)
        bv_sb = const.tile([128, 1], F32)
        bo_sb = const.tile([1, D], F32)
        bo_bc = const.tile([128, D], F32)  # bo broadcast to all partitions
        wo_sb = const.tile([128, 8 * D], BF16)  # col 1024c+n <- wo[128c+p, n]
        tri2_sb = const.tile([128, 256], BF16)
        eye_sb = const.tile([128, 128], BF16)
        xt_sb = xtp.tile([128, 8 * TI], BF16)  # col 4096g + 512c + t (g=token block)

        def load_xt_block(blk8):
            nc.sync.dma_start(
                xt_sb[:, TI * blk8 : TI * (blk8 + 1)], xt[blk8]
            )

        for w_sb, w in ((wq_sb, wq), (wk_sb, wk), (wv_sb, wv)):
            nc.sync.dma_start(w_sb[:], w[:])
        load_xt_block(0)
        nc.sync.dma_start(bq_sb[:], bq[:])
        nc.sync.dma_start(bk_sb[:], bk[:])
        nc.sync.dma_start(bv_sb[:], bv[:])
        nc.sync.dma_start(eye_sb[:], eye[:])
        load_xt_block(1)
        nc.sync.dma_start(tri2_sb[:], tri2[:])
        load_xt_block(2)
        load_xt_block(3)
        for blk8 in range(4, 8):
            load_xt_block(blk8)
        nc.sync.dma_start(bo_sb[:], bo[:])
        nc.gpsimd.partition_broadcast(bo_bc[:], bo_sb[:])
        nc.sync.dma_start(wo_sb[:], wo[:])

        # Q^T | K^T packed: col t -> Q^T, col TI + t -> K^T  (channels on partitions)
        qkt_sb = qkvp.tile([128, 2 * TI], BF16)
        # V' : [kpos(128), 32 ktiles x (2 heads x 128)]; col 256*kt + 128*h + d.
        # d 0:64 are ones columns: the AV matmul then emits the softmax
        # denominator already replicated on partitions 0:64 (no partition
        # broadcast needed, keeps the Pool queue free for the collectives);
        # d 64:128 are the V values.
        vp_sb = qkvp.tile([128, 32 * 256], BF16)

        # two half-sized AllToAlls: b=0 slices, then b=1 slices.
        # rows 256*j + 128*sl + 64*h + p  (slice s=2j+sl -> dest core s)
        a2a_in = [
            dram.tile([1024, 256], BF16, name=f"a2a_in{b}", tag=f"a2a_in{b}")
            for b in range(2)
        ]
        a2a_out = [
            dram.tile([1024, 256], BF16, name=f"a2a_out{b}", tag=f"a2a_out{b}")
            for b in range(2)
        ]

        # ---- phase A/B: QKV projections, per 512-token block.
        # Generator of PE-sized chunks so batch 1's projection work can be
        # interleaved into batch 0's attention emission.
        def ab_block(b, blk):
            base = 2048 * b + 512 * blk
            g = 4 * b + blk

            def proj(w_sb, b_sb, dst):
                ps = psS.tile([128, 512], F32, name="ps_p", tag="pss")
                for c in range(8):
                    nc.tensor.matmul(
                        ps[:],
                        w_sb[:, 128 * c : 128 * (c + 1)],
                        xt_sb[:, 4096 * g + 512 * c : 4096 * g + 512 * (c + 1)],
                        start=(c == 0),
                        stop=(c == 7),
                    )
                # evacuate with bias folded in (per-partition scalar add)
                nc.vector.tensor_scalar_add(dst, ps[:], b_sb[:])

            proj(wq_sb, bq_sb, qkt_sb[:, base : base + 512])
            yield
            proj(wk_sb, bk_sb, qkt_sb[:, TI + base : TI + base + 512])
            yield
            vt_blk = vtb.tile([128, 512], BF16, name="vt_blk")
            proj(wv_sb, bv_sb, vt_blk[:])
            yield
            # V' tiles via PE transpose
            ps2 = psS.tile([128, 512], BF16, name="ps_vt", tag="pss")
            for i in range(4):
                nc.tensor.transpose(
                    ps2[:, 128 * i : 128 * (i + 1)],
                    vt_blk[:, 128 * i : 128 * (i + 1)],
                    eye_sb[:],
                )
            kt0 = 16 * b + 4 * blk
            blk_vp = vp_sb[:, 256 * kt0 : 256 * (kt0 + 4)].rearrange(
                "p (kt h d) -> p kt h d", kt=4, h=2, d=128
            )
            nc.vector.memset(blk_vp[:, :, :, 0:64], 1.0)
            src = ps2[:].rearrange("p (i h d) -> p i h d", i=4, h=2, d=64)
            nc.vector.tensor_copy(blk_vp[:, :, :, 64:128], src)
            yield

        # batch 0 projections emitted up front; batch 1 paced into phase C
        for blk in range(4):
            for _ in ab_block(0, blk):
                pass
        ab1 = itertools.chain.from_iterable(ab_block(1, blk) for blk in range(4))

        # ---- out-projection pass for one 256-token half (after a2a b) ------------
        def outproj_pass(b):
            # out-proj DMAs ride the Activation HWDGE queue: its rings are
            # disjoint from the ones the collectives' barrier semaphores
            # cover, so these transfers never gate an AllToAll doorbell
            attn2 = aoutp.tile([128, 8 * 256], BF16, name="attn2")  # col 256c+t
            for c in range(8):
                nc.scalar.dma_start(
                    attn2[:, 256 * c : 256 * (c + 1)],
                    a2a_out[b][128 * c : 128 * (c + 1), :],
                )
            for mt in range(2):
                for nh in range(2):
                    sl = slice(512 * nh, 512 * (nh + 1))
                    po = psB.tile([128, 512], F32, name="ps_o", tag="av")
                    for c in range(8):
                        nc.tensor.matmul(
                            po[:],
                            attn2[:, 256 * c + 128 * mt : 256 * c + 128 * (mt + 1)],
                            wo_sb[:, 1024 * c + 512 * nh : 1024 * c + 512 * (nh + 1)],
                            start=(c == 0),
                            stop=(c == 7),
                        )
                    o_t = osb.tile([128, 512], F32, name="o_t")
                    nc.vector.tensor_add(o_t[:], po[:], bo_bc[:, sl])
                    nc.scalar.dma_start(
                        out[256 * b + 128 * mt : 256 * b + 128 * (mt + 1), sl], o_t[:]
                    )
                    yield

        # ---- phase C: attention; two (batch, q-block) groups in flight -----------
        class Group:
            def __init__(self, b, j):
                self.b, self.j = b, j
                self.nkb = 4 * j + 4
                self.kb_s = 0  # next k-block to score
                self.kb_a = 0  # next k-block to accumulate into AV
                self.qbase = 2048 * b + 512 * j
                self.avs = [
                    psB.tile([128, 512], F32, name=f"av{h}", tag="av")
                    for h in range(2)
                ]
                self.pts = {}

            def emit_scores(self):
                kb = self.kb_s
                self.kb_s += 1
                m = kb - 4 * self.j
                off = 128 * m if m >= 0 else 0
                kbase = TI + 2048 * self.b + 128 * kb
                pt = ptp.tile([128, 1024], BF16, name="pt")
                ps_s = psS.tile([128, 1024], F32, name="ps_s", tag="pss")
                for h in range(2):
                    hr = 64 * h
                    nc.tensor.matmul(
                        ps_s[:, 512 * h + off : 512 * (h + 1)],
                        qkt_sb[hr : hr + 64, kbase : kbase + 128],
                        qkt_sb[hr : hr + 64, self.qbase + off : self.qbase + 512],
                        start=True,
                        stop=True,
                    )
                # one Exp covers both heads (strided view over the 2 banks)
                pt_v = pt.rearrange("p (s t) -> p s t", s=2)[:, :, off:512]
                ps_v = ps_s.rearrange("p (s t) -> p s t", s=2)[:, :, off:512]
                nc.scalar.activation(
                    pt_v, ps_v, mybir.ActivationFunctionType.Exp, scale=SCALE
                )
                if m >= 0:
                    # zero the strictly-upper triangle of the diagonal
                    # 128x128 tile (both heads in one DVE op)
                    ptd = pt.rearrange("p (s t) -> p s t", s=2)[:, :, off : off + 128]
                    nc.vector.tensor_mul(
                        ptd, ptd, tri2_sb[:].rearrange("p (s t) -> p s t", s=2)
                    )
                self.pts[kb] = (pt, off)

            def emit_av(self):
                kb = self.kb_a
                self.kb_a += 1
                pt, off = self.pts.pop(kb)
                vb = 256 * (16 * self.b + kb)
                for h in range(2):
                    nc.tensor.matmul(
                        self.avs[h][:, off:],
                        vp_sb[:, vb + 128 * h : vb + 128 * h + 128],
                        pt[:, 512 * h + off : 512 * (h + 1)],
                        start=(kb == 0),
                        stop=(kb == self.nkb - 1),
                    )

            def finalize(self):
                for h in range(2):
                    # avs rows 0:64 hold the denominator replicated (ones
                    # columns of V'); base partition 0 so the custom DVE
                    # reciprocal reads the right partitions
                    rec = rp.tile([64, 512], F32, name="rec")
                    nc.vector.reciprocal_approx_fast(rec[:], self.avs[h][0:64, :])
                    at = atp.tile([64, 512], BF16, name="at")
                    nc.vector.tensor_mul(at[:], self.avs[h][64:128, :], rec[:])
                    dst = a2a_in[self.b].rearrange(
                        "(j sl h p) q -> j h p sl q", j=4, sl=2, h=2, p=64
                    )[self.j, h]
                    nc.sync.dma_start(
                        dst, at[:].rearrange("p (sl q) -> p sl q", sl=2)
                    )

        def emit_a2a(b):
            nc.gpsimd.collective_compute(
                "AllToAll",
                mybir.AluOpType.bypass,
                replica_groups=[list(range(NC))],
                ins=[a2a_in[b].opt()],
                outs=[a2a_out[b].opt()],
            )

        # long groups first so two groups stay in flight most of the time
        queue = [(0, 3), (0, 2), (0, 1), (0, 0), (1, 3), (1, 2), (1, 1), (1, 0)]
        active = []
        state = {"ab1_done": False, "b0_left": 4}

        def pace_ab1(n=1):
            if state["ab1_done"]:
                return
            for _ in range(n):
                if next(ab1, "end") == "end":
                    state["ab1_done"] = True
                    return

        while queue or active:
            while len(active) < 2 and queue:
                if queue[0][0] == 1:
                    pace_ab1(1000)  # batch-1 group: its inputs must be emitted
                g = Group(*queue.pop(0))
                g.emit_scores()
                active.append(g)
            for g in list(active):
                if g.kb_s < g.nkb:
                    g.emit_scores()
                pace_ab1(1)
                # AV trails scores by 2 k-blocks so the exp always has slack
                # before the PE consumes it (drain once scores are exhausted)
                if g.kb_a < g.kb_s - 1 or (g.kb_s == g.nkb and g.kb_a < g.nkb):
                    g.emit_av()
                if g.kb_a == g.nkb:
                    g.finalize()
                    active.remove(g)
                    if g.b == 0:
                        state["b0_left"] -= 1
                        if state["b0_left"] == 0:
                            emit_a2a(0)  # overlaps batch-1 attention
        pace_ab1(1000)

        # ---- tail: emit the second reshard FIRST (its queue-counter waits must
        # not include pass-0's DMAs), then pass-0 compute overlaps its transfer
        emit_a2a(1)
        for _ in outproj_pass(0):
            pass
        for _ in outproj_pass(1):
            pass

    nc.compile()
    return nc


_NC_CACHE = None


def _get_nc():
    global _NC_CACHE
    if _NC_CACHE is None:
        _NC_CACHE = build_nc()
    return _NC_CACHE


def _b16(a):
    return np.ascontiguousarray(np.asarray(a, np.float32).astype(ml_dtypes.bfloat16))


def make_in_maps(x, Wq, bq, Wk, bk, Wv, bv, Wo, bo):
    xf = np.asarray(x, np.float32).reshape(TI, D)
    # [g, p, c, t]: xt[g][p, 512c+t] = x[512g+t, 128c+p]
    xt = _b16(
        xf.reshape(8, 512, 8, 128).transpose(0, 3, 2, 1).reshape(8, 128, TI)
    )
    Wq, Wk, Wv, Wo = _b16(Wq), _b16(Wk), _b16(Wv), _b16(Wo)
    bof = np.ascontiguousarray(np.asarray(bo, np.float32).reshape(1, D))
    bqf = np.asarray(bq, np.float32).reshape(D, 1)
    bkf = np.asarray(bk, np.float32).reshape(D, 1)
    bvf = np.asarray(bv, np.float32).reshape(D, 1)
    # 0/1 keep mask (1 where k<=q), duplicated for both heads
    tri01 = np.where(np.arange(128)[:, None] <= np.arange(128)[None, :], 1.0, 0.0)
    tri2 = np.ascontiguousarray(
        np.concatenate([tri01, tri01], axis=1).astype(ml_dtypes.bfloat16)
    )
    eye = np.eye(128, dtype=ml_dtypes.bfloat16)
    def pack_w(W):  # [1024, 128] -> [128, 1024]: out[p, 128c+m] = W[128c+p, m]
        return np.ascontiguousarray(
            W.reshape(8, 128, 128).transpose(1, 0, 2).reshape(128, 1024)
        )

    wo_p = np.ascontiguousarray(  # [128, 8192]: out[p, 1024c+n] = Wo[128c+p, n]
        Wo.reshape(8, 128, 1024).transpose(1, 0, 2).reshape(128, 8192)
    )
    in_maps = []
    for r in range(NC):
        ch = slice(128 * r, 128 * (r + 1))
        in_maps.append(
            {
                "xt": xt,
                "wq": pack_w(Wq[:, ch]),
                "wk": pack_w(Wk[:, ch]),
                "wv": pack_w(Wv[:, ch]),
                "bq": np.ascontiguousarray(bqf[ch]),
                "bk": np.ascontiguousarray(bkf[ch]),
                "bv": np.ascontiguousarray(bvf[ch]),
                "wo": wo_p,
                "bo": bof,
                "tri2": tri2,
                "eye": eye,
            }
        )
    return in_maps


def assemble(results):
    out = np.empty((B, T, D), np.float32)
    for r in range(NC):
        res = results[r]["out"]
        out[0, 256 * r : 256 * (r + 1), :] = res[0:256]
        out[1, 256 * r : 256 * (r + 1), :] = res[256:512]
    return out


def run(inputs, trace=False, **kw):
    nc = _get_nc()
    in_maps = make_in_maps(**inputs)
    res = run_bass_kernel_spmd(nc, in_maps, core_ids=list(range(NC)), trace=trace, **kw)
    return assemble(res.results), res


def kernel(**inputs) -> np.ndarray:
    out, _ = run(inputs)
    return out
